# revision 8
# baseline (speedup 1.0000x reference)
"""Trainium2 Bass kernel for masked attention scoring (sparse_attention).

Computes, per batch b:
    proj = y @ M^T                      # [B, D]
    eij  = tanh(einsum('bsd,bd->bs', x, proj))
    a    = exp(eij) * mask
    a    = a / (sum_s a + EPS)

Sharding: data-parallel over batch B=32 across 8 NeuronCores (4 batches
per core). M is replicated; all reductions stay local per shard.

Design (memory-bound; the x f16 stream is the roofline):
  - All HWDGE input DMAs go through the single sync-engine ring, so
    completion order == issue order: M / y / mask first (small, needed
    for projT), then x per batch in two 2 MB halves, with the final
    batch's tail split off so the last-arriving piece is tiny.
  - x ships fully host-TRANSPOSED ([e, s] with d-rows pairwise
    interleaved -> 8 KB descriptors).  TensorE consumes everything:
    per (batch, s-chunk j) two contiguous 4-matmul PSUM accumulation
    groups (one per e-chunk half / DMA piece, each in its own bank -
    open accumulation groups must not interleave within a bank), then
    an ACT copy + single-PSUM-operand DVE add merges the banks.
  - projT[p, ec, b] = proj[b, ec*128+p] is computed directly on PE
    (lhsT = M^T chunk, rhs = yT) - no transposes, no broadcast.
  - Per-batch epilogue: tanh+exp (ACT) read PSUM directly, one fused
    DVE mask-mul + row-accum produces au and cs, then sum+broadcast in
    ONE matmul (lhsT=ones[P,J] replicates the total onto J partitions),
    +eps, reciprocal, PE transpose of unnormalized au, normalization as
    a per-partition ACT scale on the final copy.  Out-DMAs ride the
    second HWDGE ring (scalar engine) so they skip the x backlog.
  - The last batch's epilogue is split at j=JSPLIT so nearly all of its
    tail work completes before the final 256 KB x piece lands.
"""

import os
import sys

import numpy as np

for _p in ("/opt/trn_rl_repo",):
    if os.path.isdir(_p) and _p not in sys.path:
        sys.path.insert(0, _p)

B, S, D = 32, 2048, 1024
NCORES = 8
BL = B // NCORES        # batches per core
P = 128                 # SBUF partitions
J = S // P              # 16 s-chunks of 128 per batch
DC = D // P             # 8 e-chunks of 128
DC2 = DC // 2           # pairwise-interleaved d-row groups
JSPLIT = 12             # last batch: epilogue split point (j chunks)
EPS = 1e-7

_CACHE = {}


def _build():
    import concourse.bacc as bacc
    import concourse.tile as tile
    from concourse import mybir
    from concourse.masks import make_identity

    f32 = mybir.dt.float32
    f16 = mybir.dt.float16

    nc = bacc.Bacc("TRN2", target_bir_lowering=False, debug=False,
                   num_devices=NCORES)

    # host layout [b, dc2, p, two, s]: row (b,dc2,p) is 8 KB contiguous
    xt_ext = nc.dram_tensor("xt16", [BL, DC2, P, 2, S], f16,
                            kind="ExternalInput").ap()
    # host layout [p, dc*BL + b]: 64 B per partition row
    y_ext = nc.dram_tensor("yT16", [P, DC * BL], f16,
                           kind="ExternalInput").ap()
    # M^T pre-transposed + pairwise row-interleaved (4 KB descriptors):
    # host row q=dc2*256+2p+t holds M^T row d=dc2*256+t*128+p
    m_ext = nc.dram_tensor("MT16", [D, D], f16, kind="ExternalInput").ap()
    mk_ext = nc.dram_tensor("maskT", [P, BL, J], f32,
                            kind="ExternalInput").ap()
    out_ext = nc.dram_tensor("out", [BL, S], f32, kind="ExternalOutput").ap()

    with tile.TileContext(nc) as tc:
        with (
            tc.tile_pool(name="consts", bufs=1) as consts,
            tc.tile_pool(name="psum_proj", bufs=1, space="PSUM") as psum_proj,
            tc.tile_pool(name="psum_eij", bufs=1, space="PSUM") as psum_eij,
            tc.tile_pool(name="psum_eij2", bufs=1, space="PSUM") as psum_eij2,
            tc.tile_pool(name="psum_small", bufs=1, space="PSUM") as psum_small,
        ):
            # ---- input DMAs: small tensors first (projT dependencies),
            # then x per batch; the sync HWDGE ring is FIFO so this is
            # also the arrival order.  Last piece is tiny by design.
            mtsb = consts.tile([P, DC2, 2, D], f16)
            nc.sync.dma_start(
                out=mtsb,
                in_=m_ext.rearrange("(dc2 p two) e -> p dc2 two e",
                                    p=P, two=2))
            yT = consts.tile([P, DC, BL], f16)
            nc.sync.dma_start(
                out=yT, in_=y_ext.rearrange("p (dc b) -> p dc b", b=BL))
            mask_all = consts.tile([P, BL, J], f32)
            nc.sync.dma_start(out=mask_all, in_=mk_ext)

            xt_tiles = []
            for b in range(BL):
                xt_tiles.append(consts.tile([P, DC2, 2, S], f16,
                                            name=f"xt{b}"))
            for b in range(BL):
                src = xt_ext[b].rearrange("dc2 p two s -> p dc2 two s")
                nc.sync.dma_start(out=xt_tiles[b][:, 0:2, :, :],
                                  in_=src[:, 0:2, :, :])
                if b < BL - 1:
                    nc.sync.dma_start(out=xt_tiles[b][:, 2:, :, :],
                                      in_=src[:, 2:, :, :])
                else:
                    # align pieces with the phase-B j columns so the
                    # last-arriving piece is small and feeds the
                    # shortest dependent chain (split per dc2: the DMA
                    # balancer handles at most 3 non-trivial dims)
                    for dc2 in (2, 3):
                        nc.sync.dma_start(
                            out=xt_tiles[b][:, dc2:dc2 + 1, :,
                                            0:JSPLIT * P],
                            in_=src[:, dc2:dc2 + 1, :, 0:JSPLIT * P])
                    for dc2 in (2, 3):
                        nc.sync.dma_start(
                            out=xt_tiles[b][:, dc2:dc2 + 1, :,
                                            JSPLIT * P:],
                            in_=src[:, dc2:dc2 + 1, :, JSPLIT * P:])

            # ---- constants ----
            identity32 = consts.tile([P, P], f32)
            make_identity(nc, identity32)
            ones_pj = consts.tile([P, J], f32)
            nc.vector.memset(ones_pj, 1.0)

            # ---- projT[p, ec, b] = sum_d M[ec*128+p, d] y[b, d] ----
            projT_ps = psum_proj.tile([P, DC, BL], f32)
            for ec in range(DC):
                for dc in range(DC):
                    nc.tensor.matmul(
                        projT_ps[:, ec, :],
                        lhsT=mtsb[:, dc // 2, dc % 2,
                                  ec * P:(ec + 1) * P],
                        rhs=yT[:, dc, :],
                        start=(dc == 0),
                        stop=(dc == DC - 1),
                    )
            projT = consts.tile([P, DC, BL], f16)
            nc.scalar.copy(projT, projT_ps)

            # ---- main pass tiles ----
            eij_a = consts.tile([P, BL, J], f32)
            eij_all = consts.tile([P, BL, J], f32)
            th = consts.tile([P, BL, J], f32)
            ex = consts.tile([P, BL, J], f32)
            au = consts.tile([P, BL, J], f32)
            cs = consts.tile([P, BL], f32)
            csb = consts.tile([P, 1], f32)

            def mm(ps, xt, b, ec, j, e0, e1):
                nc.tensor.matmul(
                    ps[:, j:j + 1],
                    lhsT=xt[:, ec // 2, ec % 2, j * P:(j + 1) * P],
                    rhs=projT[:, ec, b:b + 1],
                    start=(ec == e0),
                    stop=(ec == e1 - 1),
                )

            def epilogue(ps2, b, j0, j1, cs_col):
                nc.vector.tensor_add(eij_all[:, b, j0:j1],
                                     eij_a[:, b, j0:j1], ps2[:, j0:j1])
                nc.scalar.activation(th[:, b, j0:j1], eij_all[:, b, j0:j1],
                                     mybir.ActivationFunctionType.Tanh)
                nc.scalar.activation(ex[:, b, j0:j1], th[:, b, j0:j1],
                                     mybir.ActivationFunctionType.Exp)
                nc.vector.scalar_tensor_tensor(
                    out=au[:, b, j0:j1],
                    in0=ex[:, b, j0:j1],
                    scalar=1.0,
                    in1=mask_all[:, b, j0:j1],
                    op0=mybir.AluOpType.mult,
                    op1=mybir.AluOpType.mult,
                    accum_out=cs_col,
                )

            def finishing(b, cs_cols):
                # total-sum replicated onto J partitions in one matmul
                tot_ps = psum_small.tile([J, 1], f32, tag="tot")
                for i, col in enumerate(cs_cols):
                    nc.tensor.matmul(tot_ps, lhsT=ones_pj, rhs=col,
                                     start=(i == 0),
                                     stop=(i == len(cs_cols) - 1))
                at_ps = psum_small.tile([J, P], f32, tag="attr")
                nc.tensor.transpose(at_ps, au[:, b, :], identity32)
                rec = consts.tile([J, 1], f32, name=f"rec{b}")
                nc.vector.tensor_scalar_add(rec, tot_ps, EPS)
                nc.vector.reciprocal(rec, rec)
                an_t = consts.tile([J, P], f32, name=f"ant{b}")
                nc.scalar.activation(an_t, at_ps,
                                     mybir.ActivationFunctionType.Copy,
                                     scale=rec)
                # out-DMAs ride the scalar HWDGE ring: independent FIFO,
                # and the issue directly follows the scale-copy in the
                # ACT queue (no cross-engine hop).
                nc.scalar.dma_start(
                    out=out_ext[b].rearrange("(j p) -> j p", p=P),
                    in_=an_t)

            EH = DC // 2
            for b in range(BL):
                xt = xt_tiles[b]
                ps1 = psum_eij.tile([P, J], f32, tag="eijA")
                ps2 = psum_eij2.tile([P, J], f32, tag="eijB")
                # phase A: per-column contiguous 4-matmul groups, bank A
                for j in range(J):
                    for ec in range(EH):
                        mm(ps1, xt, b, ec, j, 0, EH)
                nc.scalar.copy(eij_a[:, b, :], ps1)
                # phase B: bank B
                if b < BL - 1:
                    for j in range(J):
                        for ec in range(EH, DC):
                            mm(ps2, xt, b, ec, j, EH, DC)
                    epilogue(ps2, b, 0, J, cs[:, b:b + 1])
                    finishing(b, [cs[:, b:b + 1]])
                else:
                    for j in range(JSPLIT):
                        for ec in range(EH, DC):
                            mm(ps2, xt, b, ec, j, EH, DC)
                    epilogue(ps2, b, 0, JSPLIT, cs[:, b:b + 1])
                    for j in range(JSPLIT, J):
                        for ec in range(EH, DC):
                            mm(ps2, xt, b, ec, j, EH, DC)
                    epilogue(ps2, b, JSPLIT, J, csb)
                    finishing(b, [cs[:, b:b + 1], csb])

    nc.compile()
    return nc


def _get_nc():
    if "nc" not in _CACHE:
        _CACHE["nc"] = _build()
    return _CACHE["nc"]


def _in_maps(x, y, mask, M):
    x16 = np.asarray(x, dtype=np.float32).astype(np.float16)
    y16 = np.asarray(y, dtype=np.float32).astype(np.float16)
    MT16 = np.asarray(M, dtype=np.float32).astype(np.float16).T
    # interleave d-rows pairwise: row q=dc2*256+2p+t holds d=dc2*256+t*128+p
    MT16 = np.ascontiguousarray(
        MT16.reshape(DC2, 2, P, D).transpose(0, 2, 1, 3).reshape(D, D))
    mk = np.asarray(mask, dtype=np.int32).astype(np.float32)
    maps = []
    for i in range(NCORES):
        xs = x16[i * BL:(i + 1) * BL]                      # [BL, S, D]
        xt = xs.transpose(0, 2, 1)                         # [BL, D, S]
        xt = np.ascontiguousarray(
            xt.reshape(BL, DC2, 2, P, S).transpose(0, 1, 3, 2, 4))
        ys = y16[i * BL:(i + 1) * BL]                      # [BL, D]
        yT = np.ascontiguousarray(
            ys.T.reshape(DC, P, BL).transpose(1, 0, 2).reshape(P, DC * BL))
        mkc = mk[i * BL:(i + 1) * BL].reshape(BL, J, P).transpose(2, 0, 1)
        maps.append({
            "xt16": xt,
            "yT16": yT,
            "MT16": MT16,
            "maskT": np.ascontiguousarray(mkc),
        })
    return maps


def kernel(x, y, mask, M, **_ignored):
    from concourse.bass_utils import run_bass_kernel_spmd

    nc = _get_nc()
    res = run_bass_kernel_spmd(nc, _in_maps(x, y, mask, M),
                               core_ids=list(range(NCORES)))
    out = np.concatenate([res.results[i]["out"] for i in range(NCORES)],
                         axis=0)
    return out.astype(np.float32)
